# revision 1
# baseline (speedup 1.0000x reference)
"""
MinibatchDiscrimination kernel for 8x TRN2 NeuronCores (Bass/Tile).

Math:  x = inputs @ T  -> [B, K, D] with B=512, K=100, D=5
       out[i,k] = sum_j exp(-sum_d |x[i,k,d]-x[j,k,d]|)

Strategy (per core c of 8):
  - Host passes, per core: inputsT_c = (roll(inputs, -64c, axis=0)).T as fp16
    [F, B], T as fp16 [F, KD], plus small constant matrices. Rolling the
    batch axis makes the program SPMD-identical: every core computes output
    rows for "columns 0..63" of its own xT.
  - Device: xT[kd, i] = sum_f T[f, kd] * inputsT[f, i]   (PE, 4 chunks of 125)
    S[k, i] = sum_d x[i,k,d]  (PE ones-block matmul over xT, stored fp16)
  - Per output row j in 0..63, using |t| = 2*relu(t) - t:
      rl_c[p,i]   = relu(xT_c[p,i] - xT_c[p,j])   (DVE tensor_scalar
                                                   (subtract, max 0.0); the
                                                   per-partition scalar is an
                                                   f32 upcast of the fp16 xT
                                                   column so the diagonal is
                                                   exactly 0)
      dist[:, i]  = -S[k,i] + 2*sum_d rl           (PE: negI matmul into psum +
                                                    2.0-block col-tiled matmuls)
      raw[:, j]   = sum_i exp(-dist[:,i])          (ACT fused exp + accum_out,
                                                    no bias)
      out         = raw * exp(-S16[:, 0:64])       (one DVE multiply at the
                                                    end: the per-partition
                                                    exp(-S_kj) factor is
                                                    constant over i, so it
                                                    factors out of the sum)
    since sum_d |diff| = 2*sum_d relu(diff) - S_ki + S_kj, and the S terms
    cancel exactly on the diagonal.
  - dist row p=32c+m holds k=25c+m (m<25); host transposes/reassembles.

  Hardware notes baked into the structure (measured on TRN2):
  - Compute instructions carry at most ONE semaphore wait after bacc's
    split pass; persistent manually-rotated tiles (dist/dump/ab) keep
    cross-iteration WAR deps same-engine so waits stay within budget.
  - The pipeline is ACT/DVE-bound: exp+accum ~= 0.85us per row; DVE
    (4 tensor_scalars ~1us) and PE (~0.9us, partially col-tile-overlapped)
    overlap with it. Measured steady state ~64-72us for the 64-row loop
    (axon-link wall-clock noise is +-10us; best measured 63.8us).
    GPSIMD offload of a relu chunk was tried and is ~7x slower than DVE
    on the Q7 path -- do not route tensor_scalar to gpsimd here.
    Also measured as no-wins (within +-10us link noise): deeper ab/dist
    buffering (12/5), exp main-out to SBUF fp16 instead of PSUM f32,
    and a rank-1 PE matmul replacing the exp bias (that one regressed
    ~40% -- K=1 matmuls serialize on the PE critical path).
  - Residual overhead, quantified from the final IR: each relu
    tensor_scalar carries a redundant same-engine WAW wait (ab-buffer
    rotation) in addition to its real PE WAR; bacc's 1-wait limit splits
    it into an EventSemaphore on the DVE queue -- 242 of them, ~10us of
    issue time on the bottleneck engine. Eliding same-engine WAW sems in
    Tile/bacc would recover most of the gap to the ~60us arithmetic floor.
  - The input stage (DMA 2MB fp16, xT matmuls, S row-sums) overlaps the
    loop start; the ACT exp table is pre-warmed during the DMAs.

By symmetry of the distance matrix, summing exp(-dist) over the free axis i
for a fixed row j gives exactly out[j, k] (self term included).
"""

import sys
import numpy as np

for _p in ("/opt/trn_rl_repo",):
    if _p not in sys.path:
        sys.path.insert(0, _p)

B = 512
F = 1024
K = 100
D = 5
KD = K * D  # 500
NCORES = 8
JPC = B // NCORES  # 64 output rows per core
NCHUNK = 4  # kd chunks of 125
CHUNK = KD // NCHUNK  # 125
KPC = K // NCHUNK  # 25 k's per chunk

_NC_CACHE = {}


def build_nc(bench_reps=1, ablate=()):
    import contextlib

    import concourse.bass as bass
    import concourse.bacc as bacc
    import concourse.mybir as mybir
    from concourse.tile import TileContext

    nc = bacc.Bacc(None, target_bir_lowering=False, debug=True)

    inT = nc.declare_dram_parameter("inT", [F, B], mybir.dt.float16, isOutput=False)
    Tm = nc.declare_dram_parameter("Tm", [F, KD], mybir.dt.float16, isOutput=False)
    # [:, 0:32] 2.0-valued d-sum block, [:, 32:64] 1.0-valued d-sum block
    onesd = nc.declare_dram_parameter(
        "onesd", [CHUNK, 64], mybir.dt.float16, isOutput=False
    )
    negI = nc.declare_dram_parameter("negI", [128, 128], mybir.dt.float16, isOutput=False)
    out = nc.declare_dram_parameter("out", [128, JPC], mybir.dt.float32, isOutput=True)

    with TileContext(nc) as tc:
        with tc.tile_pool(name="persist", bufs=1) as pp:
            T_sb = pp.tile([128, 8 * KD], mybir.dt.float16, name="T_sb")
            inT_sb = pp.tile([128, 8 * B], mybir.dt.float16, name="inT_sb")
            ones_sb = pp.tile([CHUNK, 64], mybir.dt.float16, name="ones_sb")
            negI_sb = pp.tile([128, 128], mybir.dt.float16, name="negI_sb")
            out_sb = pp.tile([128, JPC], mybir.dt.float32, name="out_sb")
            xT_sb = pp.tile([128, NCHUNK * B], mybir.dt.float16, name="xT_sb")
            # f32 upcasts of xT columns 0..JPC (tensor_scalar per-partition
            # scalars must be f32). Upcast from the fp16 xT so the diagonal
            # max(x,x)-x stays exactly zero.
            xTj_sb = pp.tile([128, NCHUNK * JPC], mybir.dt.float32, name="xTj_sb")
            S16_sb = pp.tile([128, B], mybir.dt.float16, name="S16_sb")
            expS_sb = pp.tile([128, JPC], mybir.dt.float32, name="expS_sb")
            raw_sb = pp.tile([128, JPC], mybir.dt.float32, name="raw_sb")

            # warm the ACT exp table while DMAs run (table load ~2.7us)
            warm_sb = pp.tile([1, 1], mybir.dt.float32, name="warm_sb")
            nc.vector.memset(warm_sb[:, :], 0.0)
            nc.scalar.activation(
                warm_sb[:, :], warm_sb[:, :], mybir.ActivationFunctionType.Exp
            )

            # --- load inputs ---
            for t in range(8):
                nc.sync.dma_start(
                    out=T_sb[:, t * KD : (t + 1) * KD],
                    in_=Tm[t * 128 : (t + 1) * 128, :],
                )
                nc.sync.dma_start(
                    out=inT_sb[:, t * B : (t + 1) * B],
                    in_=inT[t * 128 : (t + 1) * 128, :],
                )
            nc.sync.dma_start(out=ones_sb[:, :], in_=onesd[:, :])
            nc.sync.dma_start(out=negI_sb[:, :], in_=negI[:, :])

            with tc.tile_pool(name="xtps", bufs=2, space="PSUM") as xtps:
                # --- xT chunks: xT[kd, i] via PE over f tiles ---
                for c in range(NCHUNK):
                    xt_ps = xtps.tile([CHUNK, B], mybir.dt.float32, name="xt_ps")
                    for t in range(8):
                        nc.tensor.matmul(
                            xt_ps[:, :],
                            T_sb[:, t * KD + c * CHUNK : t * KD + (c + 1) * CHUNK],
                            inT_sb[:, t * B : (t + 1) * B],
                            start=(t == 0),
                            stop=(t == 7),
                        )
                    nc.scalar.copy(xT_sb[0:CHUNK, c * B : (c + 1) * B], xt_ps[:, :])
                    nc.vector.tensor_copy(
                        xTj_sb[0:CHUNK, c * JPC : (c + 1) * JPC],
                        xT_sb[0:CHUNK, c * B : c * B + JPC],
                    )

                # --- S[k, i] = sum_d x[i,k,d], arranged at partitions 32c+m ---
                S_ps = xtps.tile([128, B], mybir.dt.float32, name="S_ps", bufs=1)
                for c in range(NCHUNK):
                    nc.tensor.matmul(
                        S_ps[32 * c : 32 * c + 32, :],
                        ones_sb[:, 32:64],
                        xT_sb[0:CHUNK, c * B : (c + 1) * B],
                        start=True,
                        stop=True,
                        tile_position=(0, 32 * c),
                    )
                nc.vector.tensor_copy(S16_sb[:, :], S_ps[:, :])
                # exp(-S16[:, j]) factors: the per-partition exp bias is
                # constant over i, so it moves out of the accumulated sum
                # and becomes one elementwise multiply at the end.
                nc.scalar.activation(
                    expS_sb[:, :],
                    S16_sb[:, 0:JPC],
                    mybir.ActivationFunctionType.Exp,
                    bias=0.0,
                    scale=-1.0,
                )

            mainps_es = contextlib.ExitStack()
            mainps = mainps_es.enter_context(
                tc.tile_pool(name="mainps", bufs=1, space="PSUM")
            )

            # Persistent, manually double-buffered psum tiles. Persistent
            # (vs pool-rotated) so cross-iteration WAR deps are plain data
            # deps on fixed tiles: same-engine deps then cost no semaphore,
            # which matters because instructions carry at most ONE wait.
            dist_bufs = [
                mainps.tile([128, B], mybir.dt.float32, name=f"dist{i}") for i in range(4)
            ]
            dump_bufs = [
                mainps.tile([128, B], mybir.dt.float32, name=f"dump{i}") for i in range(2)
            ]
            # Persistent relu tiles, manually rotated (same reason).
            NAB = 8
            ab_bufs = [
                pp.tile([CHUNK, B], mybir.dt.float16, name=f"ab{i}") for i in range(NAB)
            ]

            # --- main loop over output rows ---
            # bench_reps>1 wraps the j-loop in a dynamic For_i so one NEFF
            # execution repeats the steady-state body (timing harness only).
            if ablate:
                # one unablated pass so every tile has a writer
                main_loop(nc, mybir, xT_sb, xTj_sb, S16_sb, raw_sb, negI_sb,
                          ones_sb, out_sb, dist_bufs, dump_bufs, ab_bufs)
            rep_ctx = (
                tc.For_i(0, bench_reps, 1) if bench_reps > 1 else contextlib.nullcontext()
            )
            with rep_ctx:
                main_loop(nc, mybir, xT_sb, xTj_sb, S16_sb, raw_sb, negI_sb,
                          ones_sb, out_sb, dist_bufs, dump_bufs, ab_bufs, ablate)

            # out = raw_sums * exp(-S16[:, j]) (the factored-out bias)
            nc.vector.tensor_tensor(
                out_sb[:, :], raw_sb[:, :], expS_sb[:, :], mybir.AluOpType.mult
            )
            mainps_es.close()
            nc.sync.dma_start(out=out[:, :], in_=out_sb[:, :])

    nc.finalize()
    return nc


def main_loop(nc, mybir, xT_sb, xTj_sb, S16_sb, raw_sb, negI_sb, ones_sb,
              out_sb, dist_bufs, dump_bufs, ab_bufs, ablate=()):
    NAB = len(ab_bufs)
    if True:
            for j in range(JPC):
                dist = dist_bufs[j % 4]
                # dist = -S[k, i]; also the first touch of dist this
                # iteration, absorbing the WAR-vs-ACT(exp of j-2) wait.
                if "mms" not in ablate and "mm" not in ablate:
                    nc.tensor.matmul(
                        dist[:, :],
                        negI_sb[:, :],
                        S16_sb[:, :],
                        start=True,
                        stop=False,
                        skip_group_check=True,
                    )
                for c in range(NCHUNK):
                    ab = ab_bufs[(j * NCHUNK + c) % NAB]
                    # relu(x_i - x_j) = max(x_i, x_j) - x_j
                    if "ts" not in ablate:
                        # relu(x_i - x_j) = (x_i - x_j) max 0; const scalar2
                        # keeps the second DVE read port free for 2x_2p/4x.
                        s1 = (
                            0.5
                            if "tsconst" in ablate
                            else xTj_sb[0:CHUNK, c * JPC + j : c * JPC + j + 1]
                        )
                        nc.vector.tensor_scalar(
                            ab[:, :],
                            xT_sb[0:CHUNK, c * B : (c + 1) * B],
                            s1,
                            0.0,
                            mybir.AluOpType.subtract,
                            mybir.AluOpType.max,
                        )
                    # dist[32c+m, :] += 2 * sum_d ab[5m+d, :]
                    if "mm" not in ablate:
                        nc.tensor.matmul(
                            dist[32 * c : 32 * c + 32, :],
                            ones_sb[:, 0:32],
                            ab[:, :],
                            start=("mms" in ablate and c == 0),
                            stop=(c == NCHUNK - 1),
                            tile_position=(0, 32 * c),
                            skip_group_check=True,
                        )
                dump = dump_bufs[j % 2]
                # out_sb[:, j] = sum_i exp(-dist[:, i] - S16[:, j])
                if "exp" not in ablate:
                    nc.scalar.activation(
                        dump[:, :],
                        dist[:, :],
                        mybir.ActivationFunctionType.Exp,
                        bias=0.0,
                        scale=-1.0,
                        accum_out=(None if "noaccum" in ablate else raw_sb[:, j : j + 1]),
                    )


def _aux_consts():
    ob = np.zeros([CHUNK, 64], dtype=np.float16)
    for m in range(KPC):
        ob[5 * m : 5 * m + 5, m] = 2.0
        ob[5 * m : 5 * m + 5, 32 + m] = 1.0
    negI = (-np.eye(128)).astype(np.float16)
    return ob, negI


def make_in_maps(inputs, T):
    f16 = np.float16
    Tm = np.asarray(T, dtype=np.float32).astype(f16)
    ob, negI = _aux_consts()
    in_maps = []
    for c in range(NCORES):
        rolled = np.roll(np.asarray(inputs, dtype=np.float32), -JPC * c, axis=0)
        inTc = np.ascontiguousarray(rolled.T).astype(f16)
        in_maps.append(
            {
                "inT": inTc,
                "Tm": Tm,
                "onesd": ob,
                "negI": negI,
            }
        )
    return in_maps


def assemble_output(results):
    out = np.zeros([B, K], dtype=np.float32)
    for c in range(NCORES):
        arr = np.asarray(results[c]["out"], dtype=np.float32)  # [128, JPC]
        for cc in range(NCHUNK):
            out[JPC * c : JPC * (c + 1), KPC * cc : KPC * (cc + 1)] = arr[
                32 * cc : 32 * cc + KPC, :
            ].T
    return out


def kernel(inputs, T):
    from concourse.bass_utils import run_bass_kernel_spmd

    if "nc" not in _NC_CACHE:
        _NC_CACHE["nc"] = build_nc()
    nc = _NC_CACHE["nc"]
    in_maps = make_in_maps(inputs, T)
    res = run_bass_kernel_spmd(nc, in_maps, list(range(NCORES)))
    return assemble_output(res.results)


if __name__ == "__main__":
    sys.path.insert(0, "/root/problem")
    from reference import setup_inputs, reference

    inputs = setup_inputs()
    expected = np.asarray(reference(**inputs))
    actual = kernel(**{k: np.asarray(v) for k, v in inputs.items()})
    err = np.abs(actual - expected)
    rel = np.linalg.norm(actual - expected) / np.linalg.norm(expected)
    print(f"max abs err: {err.max():.3e}")
    print(f"Relative error: {rel:.3e}")



# revision 5
# speedup vs baseline: 1.3974x; 1.3974x over previous
"""
MinibatchDiscrimination kernel for 8x TRN2 NeuronCores (Bass/Tile).

Math:  x = inputs @ T  -> [B, K, D] with B=512, K=100, D=5
       out[i,k] = sum_j exp(-sum_d |x[i,k,d]-x[j,k,d]|)

Strategy — symmetric block-tournament over the pairwise matrix:

  The B x B pairwise matrix is tiled into 8x8 blocks of 64x64 (one row-group
  per core). Each unordered block-pair only needs computing once: from one
  computed block, ROW sums come from the ACT accumulator and COLUMN sums
  (= row sums of the transposed block, by symmetry of the L1 distance) come
  from a PE identity-matmul accumulation over the exp tiles. Core c computes
  blocks (c, c+k) for k=0..4 (mod 8, W=320 columns of its rolled copy):

    - diag block (k=0): row sums only (colsum would double-count by symmetry)
    - k=1,2,3: row sums kept locally + column sums exchanged to core c+k
      (exchange happens on the host during output assembly)
    - k=4: row sums only; the mirror pair {c, c+4} is computed independently
      by core c+4 as ITS k=4 block (distance-4 blocks are duplicated so the
      SPMD program stays identical across cores)

  Row j of core q then receives: own row sums (col-groups q..q+4) plus
  exchanged column sums from cores q-1, q-2, q-3 — all 8 groups exactly once.

Per core c of 8 (rolled by 64c so the program is SPMD-identical):
  - xT[kd, i] = sum_f T[f, kd] * inT[f, i] on PE (4 chunks of 125 kd), i<320.
  - Per output row j in 0..63:
      ab_c[p, i] = |xT_c[p, i] - xT_c[p, j]|   (DVE tensor_scalar
                   (subtract, abs_max vs 0.0) — fp16, 4x perf mode;
                   the per-partition scalar is an f32 upcast of the fp16 xT
                   column so the diagonal is exactly 0)
      dist[32c+m, :] = sum_d ab[5m+d, :]       (PE d-sum matmul with a
                   0/1 block matrix, col-tiled per chunk — no S-term or
                   negI matmul needed since abs values sum directly)
      dump[:, :]  = exp(-dist), fp16 -> SBUF   (ACT, accum_out gives the
                   row sums over all 320 cols in one pass)
      colacc     += dump[:, 64:256]            (PE identity matmul
                   accumulating in PSUM across all 64 j — the k=1,2,3
                   column sums, emitted 2 iterations late to pipeline)
  - dist row p=32c+m holds k=25c+m (m<25); host transposes/reassembles and
    adds the exchanged column-sum blocks.

  Hardware notes (CoreSim cost model, validated on TRN2 previously):
  - Steady state is ACT-bound: exp main pass 0.833*320+185 = 452ns plus the
    fixed 287ns accumulator-read = ~739ns/row; DVE 4x tensor_scalars at
    143.8ns = 575ns/row and PE 4 d-sums + colacc = 613ns/row overlap under
    it. 64 rows -> ~47us steady.
  - ab/dump tiles are STATIC rings sized to the whole loop (256 ab tiles,
    ~160KB of SBUF) so there are no cross-iteration WAW deps at all: DVE
    instructions carry no waits in steady state (the baseline lost ~10us+
    to 242 same-engine WAW EventSemaphores from rotating small rings).
  - Inputs land in 4 DMAs (two ~0.5-1MB strided transfers each for T/inT
    halves) so SP descriptor-gen time stays off the critical path; the ACT
    exp table is pre-warmed during the DMAs.
"""

import sys
import numpy as np

for _p in ("/opt/trn_rl_repo",):
    if _p not in sys.path:
        sys.path.insert(0, _p)

B = 512
F = 1024
K = 100
D = 5
KD = K * D  # 500
NCORES = 8
JPC = B // NCORES  # 64 output rows per core
NCHUNK = 4  # kd chunks of 125
CHUNK = KD // NCHUNK  # 125
KPC = K // NCHUNK  # 25 k's per chunk
NBLK = 5  # col block-groups computed per core (k = 0..4)
W = NBLK * JPC  # 320 pairwise columns per core
NEX = 3  # exchanged colsum groups (k = 1, 2, 3)
CEX = NEX * JPC  # 192 exchanged columns (local cols 64..256)

_NC_CACHE = {}


def build_nc():
    import contextlib

    import concourse.bass as bass
    import concourse.bacc as bacc
    import concourse.mybir as mybir
    from concourse.tile import TileContext

    nc = bacc.Bacc(None, target_bir_lowering=False, debug=True)

    inT = nc.declare_dram_parameter("inT", [F, W], mybir.dt.float16, isOutput=False)
    Tm = nc.declare_dram_parameter("Tm", [F, KD], mybir.dt.float16, isOutput=False)
    # dmat[5m+d, m] = 2.0 (d-sum of 2*relu), dmat[5m+d, 32+m] = 1.0 (S row sums)
    dmat = nc.declare_dram_parameter(
        "dmat", [CHUNK, 64], mybir.dt.float16, isOutput=False
    )
    negI = nc.declare_dram_parameter("negI", [128, 128], mybir.dt.float16, isOutput=False)
    ident = nc.declare_dram_parameter("ident", [128, 128], mybir.dt.float16, isOutput=False)
    rowsum = nc.declare_dram_parameter("rowsum", [128, JPC], mybir.dt.float32, isOutput=True)
    colout = nc.declare_dram_parameter("colout", [128, CEX], mybir.dt.float32, isOutput=True)

    with TileContext(nc) as tc:
        with tc.tile_pool(name="persist", bufs=1) as pp:
            T_sb = pp.tile([128, 8 * KD], mybir.dt.float16, name="T_sb")
            inT_sb = pp.tile([128, 8 * W], mybir.dt.float16, name="inT_sb")
            dmat_sb = pp.tile([CHUNK, 64], mybir.dt.float16, name="dmat_sb")
            negI_sb = pp.tile([128, 128], mybir.dt.float16, name="negI_sb")
            S16_sb = pp.tile([128, W], mybir.dt.float16, name="S16_sb")
            negSj_sb = pp.tile([128, JPC], mybir.dt.float32, name="negSj_sb")
            ident_sb = pp.tile([128, 128], mybir.dt.float16, name="ident_sb")
            xT_sb = pp.tile([128, NCHUNK * W], mybir.dt.float16, name="xT_sb")
            # f32 upcasts of xT columns 0..JPC (tensor_scalar per-partition
            # scalars must be f32). Upcast from the fp16 xT so the diagonal
            # |x - x| stays exactly zero.
            xTj_sb = pp.tile([128, NCHUNK * JPC], mybir.dt.float32, name="xTj_sb")
            raw_sb = pp.tile([128, JPC], mybir.dt.float32, name="raw_sb")
            colout_sb = pp.tile([128, CEX], mybir.dt.float32, name="colout_sb")

            # warm the ACT exp table while DMAs run (table load ~1.3us)
            warm_sb = pp.tile([1, 1], mybir.dt.float32, name="warm_sb")
            nc.vector.memset(warm_sb[:, :], 0.0)
            nc.scalar.activation(
                warm_sb[:, :], warm_sb[:, :], mybir.ActivationFunctionType.Exp
            )

            # Static rings: every (j, chunk) gets its own ab tile and every
            # j its own dump slot modulo 8 — cross-iteration WAW deps are
            # either absent (ab) or satisfied 8 iterations early (dump).
            ab_ring = [
                pp.tile([CHUNK, W], mybir.dt.float16, name=f"ab{t}")
                for t in range(JPC * NCHUNK)
            ]
            NDUMP = 8
            dump_ring = [
                pp.tile([128, W], mybir.dt.float16, name=f"dump{t}")
                for t in range(NDUMP)
            ]

            # --- load inputs: 2 halves each of T/inT so matmuls can start
            # after the first halves land, in 6 total strided DMAs ---
            for h in range(2):
                nc.sync.dma_start(
                    out=T_sb[:, h * 4 * KD : (h + 1) * 4 * KD].rearrange(
                        "p (t k) -> p t k", t=4
                    ),
                    in_=Tm[h * 512 : (h + 1) * 512, :].rearrange(
                        "(t p) k -> p t k", t=4
                    ),
                )
                nc.sync.dma_start(
                    out=inT_sb[:, h * 4 * W : (h + 1) * 4 * W].rearrange(
                        "p (t w) -> p t w", t=4
                    ),
                    in_=inT[h * 512 : (h + 1) * 512, :].rearrange(
                        "(t p) w -> p t w", t=4
                    ),
                )
            nc.sync.dma_start(out=dmat_sb[:, :], in_=dmat[:, :])
            nc.sync.dma_start(out=ident_sb[:, :], in_=ident[:, :])
            nc.sync.dma_start(out=negI_sb[:, :], in_=negI[:, :])

            with tc.tile_pool(name="xtps", bufs=2, space="PSUM") as xtps:
                # --- xT chunks: xT[kd, i] via PE over f tiles ---
                for c in range(NCHUNK):
                    xt_ps = xtps.tile([CHUNK, W], mybir.dt.float32, name="xt_ps")
                    for t in range(8):
                        nc.tensor.matmul(
                            xt_ps[:, :],
                            T_sb[:, t * KD + c * CHUNK : t * KD + (c + 1) * CHUNK],
                            inT_sb[:, t * W : (t + 1) * W],
                            start=(t == 0),
                            stop=(t == 7),
                        )
                    # alternate the PSUM->SBUF fp16 copies between ACT and
                    # DVE so the input stage drains faster
                    if c % 2 == 0:
                        nc.scalar.copy(xT_sb[0:CHUNK, c * W : (c + 1) * W], xt_ps[:, :])
                    else:
                        nc.vector.tensor_copy(
                            xT_sb[0:CHUNK, c * W : (c + 1) * W], xt_ps[:, :]
                        )
                    nc.vector.tensor_copy(
                        xTj_sb[0:CHUNK, c * JPC : (c + 1) * JPC],
                        xT_sb[0:CHUNK, c * W : c * W + JPC],
                    )

                # --- S[k, i] = sum_d x[i,k,d] at partitions 32c+m ---
                S_ps = xtps.tile([128, W], mybir.dt.float32, name="S_ps", bufs=1)
                for c in range(NCHUNK):
                    nc.tensor.matmul(
                        S_ps[32 * c : 32 * c + 32, :],
                        dmat_sb[:, 32:64],
                        xT_sb[0:CHUNK, c * W : (c + 1) * W],
                        start=True,
                        stop=True,
                        tile_position=(0, 32 * c),
                    )
                nc.vector.tensor_copy(S16_sb[:, :], S_ps[:, :])
                # bias column for the exp: -S_j, upcast from the SAME fp16
                # S16 used by the negI matmul so the diagonal cancels exactly
                nc.vector.tensor_scalar(
                    negSj_sb[:, :],
                    S16_sb[:, 0:JPC],
                    -1.0,
                    0.0,
                    mybir.AluOpType.mult,
                    mybir.AluOpType.bypass,
                )

            mainps_es = contextlib.ExitStack()
            mainps = mainps_es.enter_context(
                tc.tile_pool(name="mainps", bufs=1, space="PSUM")
            )
            NDIST = 6
            dist_bufs = [
                mainps.tile([128, W], mybir.dt.float32, name=f"dist{i}")
                for i in range(NDIST)
            ]
            colacc = mainps.tile([128, CEX], mybir.dt.float32, name="colacc")

            # --- main loop over output rows ---
            for j in range(JPC):
                dist = dist_bufs[j % NDIST]
                # dist = -S[k, i] (also absorbs the WAR wait vs the ACT exp
                # that last read this dist buffer)
                nc.tensor.matmul(
                    dist[:, :],
                    negI_sb[:, :],
                    S16_sb[:, :],
                    start=True,
                    stop=False,
                    skip_group_check=True,
                )
                for c in range(NCHUNK):
                    ab = ab_ring[j * NCHUNK + c]
                    # ab = relu(xT[:, i] - xT[:, j]) : (in - s1) max 0.0
                    # (const scalar2 keeps the second DVE read port free so
                    # the 4x perf mode applies)
                    nc.vector.tensor_scalar(
                        ab[:, :],
                        xT_sb[0:CHUNK, c * W : (c + 1) * W],
                        xTj_sb[0:CHUNK, c * JPC + j : c * JPC + j + 1],
                        0.0,
                        mybir.AluOpType.subtract,
                        mybir.AluOpType.max,
                    )
                    # dist[32c+m, :] += 2 * sum_d ab[5m+d, :]
                    nc.tensor.matmul(
                        dist[32 * c : 32 * c + 32, :],
                        dmat_sb[:, 0:32],
                        ab[:, :],
                        start=False,
                        stop=(c == NCHUNK - 1),
                        tile_position=(0, 32 * c),
                        skip_group_check=True,
                    )
                # dump = exp(-dist - S_j) = exp(-L1(i,j)) fp16;
                # accum_out gives the row sums
                nc.scalar.activation(
                    dump_ring[j % NDUMP][:, :],
                    dist[:, :],
                    mybir.ActivationFunctionType.Exp,
                    bias=negSj_sb[:, j : j + 1],
                    scale=-1.0,
                    accum_out=raw_sb[:, j : j + 1],
                )
                # colacc += dump[:, 64:256], two iterations late so PE never
                # waits on the ACT exp of the current row
                jl = j - 2
                if jl >= 0:
                    nc.tensor.matmul(
                        colacc[:, :],
                        ident_sb[:, :],
                        dump_ring[jl % NDUMP][:, JPC : JPC + CEX],
                        start=(jl == 0),
                        stop=False,
                        skip_group_check=True,
                    )
            for jl in (JPC - 2, JPC - 1):
                nc.tensor.matmul(
                    colacc[:, :],
                    ident_sb[:, :],
                    dump_ring[jl % NDUMP][:, JPC : JPC + CEX],
                    start=False,
                    stop=(jl == JPC - 1),
                    skip_group_check=True,
                )

            nc.scalar.copy(colout_sb[:, :], colacc[:, :])
            mainps_es.close()
            nc.sync.dma_start(out=rowsum[:, :], in_=raw_sb[:, :])
            nc.sync.dma_start(out=colout[:, :], in_=colout_sb[:, :])

    nc.finalize()
    return nc


def _aux_consts():
    dm = np.zeros([CHUNK, 64], dtype=np.float16)
    for m in range(KPC):
        dm[5 * m : 5 * m + 5, m] = 2.0
        dm[5 * m : 5 * m + 5, 32 + m] = 1.0
    ident = np.eye(128).astype(np.float16)
    negI = (-np.eye(128)).astype(np.float16)
    return dm, ident, negI


def make_in_maps(inputs, T):
    f16 = np.float16
    Tm = np.asarray(T, dtype=np.float32).astype(f16)
    dm, ident, negI = _aux_consts()
    in_maps = []
    x = np.asarray(inputs, dtype=np.float32)
    for c in range(NCORES):
        rolled = np.roll(x, -JPC * c, axis=0)[0:W, :]
        inTc = np.ascontiguousarray(rolled.T).astype(f16)
        in_maps.append(
            {
                "inT": inTc,
                "Tm": Tm,
                "dmat": dm,
                "ident": ident,
                "negI": negI,
            }
        )
    return in_maps


def assemble_output(results):
    out = np.zeros([B, K], dtype=np.float32)
    # own row sums: raw[32c+m, j] -> out[64q+j, 25c+m]
    for q in range(NCORES):
        raw = np.asarray(results[q]["rowsum"], dtype=np.float32)  # [128, JPC]
        for cc in range(NCHUNK):
            out[JPC * q : JPC * (q + 1), KPC * cc : KPC * (cc + 1)] = raw[
                32 * cc : 32 * cc + KPC, :
            ].T
    # exchanged column sums: core b's group k (k=1..3) serves rows of b+k
    for b in range(NCORES):
        col = np.asarray(results[b]["colout"], dtype=np.float32)  # [128, CEX]
        for k in range(1, NEX + 1):
            q = (b + k) % NCORES
            blk = col[:, JPC * (k - 1) : JPC * k]  # [128, JPC]
            for cc in range(NCHUNK):
                out[JPC * q : JPC * (q + 1), KPC * cc : KPC * (cc + 1)] += blk[
                    32 * cc : 32 * cc + KPC, :
                ].T
    return out


def kernel(inputs, T):
    from concourse.bass_utils import run_bass_kernel_spmd

    if "nc" not in _NC_CACHE:
        _NC_CACHE["nc"] = build_nc()
    nc = _NC_CACHE["nc"]
    in_maps = make_in_maps(inputs, T)
    res = run_bass_kernel_spmd(nc, in_maps, list(range(NCORES)))
    return assemble_output(res.results)


if __name__ == "__main__":
    sys.path.insert(0, "/root/problem")
    from reference import setup_inputs, reference

    inputs = setup_inputs()
    expected = np.asarray(reference(**inputs))
    actual = kernel(**{k: np.asarray(v) for k, v in inputs.items()})
    err = np.abs(actual - expected)
    rel = np.linalg.norm(actual - expected) / np.linalg.norm(expected)
    print(f"max abs err: {err.max():.3e}")
    print(f"Relative error: {rel:.3e}")


# revision 7
# speedup vs baseline: 1.5223x; 1.0894x over previous
"""
MinibatchDiscrimination kernel for 8x TRN2 NeuronCores (Bass/Tile).

Math:  x = inputs @ T  -> [B, K, D] with B=512, K=100, D=5
       out[i,k] = sum_j exp(-sum_d |x[i,k,d]-x[j,k,d]|)

Strategy — symmetric block-tournament over the pairwise matrix:

  The B x B pairwise matrix is tiled into 8x8 blocks of 64x64 (one row-group
  per core). Each unordered block-pair only needs computing once: from one
  computed block, ROW sums come from the ACT accumulator and COLUMN sums
  (= row sums of the transposed block, by symmetry of the L1 distance) come
  from a PE identity-matmul accumulation over the exp tiles. Core c computes
  blocks (c, c+k) for k=0..4 (mod 8, W=320 columns of its rolled copy):

    - diag block (k=0): row sums only (colsum would double-count by symmetry)
    - k=1,2,3: row sums kept locally + column sums exchanged to core c+k
      (exchange happens on the host during output assembly)
    - k=4: row sums only; the mirror pair {c, c+4} is computed independently
      by core c+4 as ITS k=4 block (distance-4 blocks are duplicated so the
      SPMD program stays identical across cores)

  Row j of core q then receives: own row sums (col-groups q..q+4) plus
  exchanged column sums from cores q-1, q-2, q-3 — all 8 groups exactly once.

Per core c of 8 (rolled by 64c so the program is SPMD-identical):
  - xT[kd, i] = sum_f T[f, kd] * inT[f, i] on PE (4 chunks of 125 kd), i<320.
  - Per output row j in 0..63:
      ab_c[p, i] = |xT_c[p, i] - xT_c[p, j]|   (DVE tensor_scalar
                   (subtract, abs_max vs 0.0) — fp16, 4x perf mode;
                   the per-partition scalar is an f32 upcast of the fp16 xT
                   column so the diagonal is exactly 0)
      dist[32c+m, :] = sum_d ab[5m+d, :]       (PE d-sum matmul with a
                   0/1 block matrix, col-tiled per chunk — no S-term or
                   negI matmul needed since abs values sum directly)
      dump[:, :]  = exp(-dist), fp16 -> SBUF   (ACT, accum_out gives the
                   row sums over all 320 cols in one pass)
      colacc     += dump[:, 64:256]            (PE identity matmul
                   accumulating in PSUM across all 64 j — the k=1,2,3
                   column sums, emitted 2 iterations late to pipeline)
  - dist row p=32c+m holds k=25c+m (m<25); host transposes/reassembles and
    adds the exchanged column-sum blocks.

  Hardware notes (CoreSim cost model, validated on TRN2 previously):
  - Steady state is ACT-bound: exp main pass 0.833*320+185 = 452ns plus the
    fixed 287ns accumulator-read = ~739ns/row; DVE 4x tensor_scalars at
    143.8ns = 575ns/row and PE 4 d-sums + colacc = 613ns/row overlap under
    it. 64 rows -> ~47us steady.
  - ab/dump tiles are STATIC rings sized to the whole loop (256 ab tiles,
    ~160KB of SBUF) so there are no cross-iteration WAW deps at all: DVE
    instructions carry no waits in steady state (the baseline lost ~10us+
    to 242 same-engine WAW EventSemaphores from rotating small rings).
  - Inputs land in 4 DMAs (two ~0.5-1MB strided transfers each for T/inT
    halves) so SP descriptor-gen time stays off the critical path; the ACT
    exp table is pre-warmed during the DMAs.
"""

import sys
import numpy as np

for _p in ("/opt/trn_rl_repo",):
    if _p not in sys.path:
        sys.path.insert(0, _p)

B = 512
F = 1024
K = 100
D = 5
KD = K * D  # 500
NCORES = 8
JPC = B // NCORES  # 64 output rows per core
NCHUNK = 4  # kd chunks of 125
CHUNK = KD // NCHUNK  # 125
KPC = K // NCHUNK  # 25 k's per chunk
NBLK = 5  # col block-groups computed per core (k = 0..4)
W = NBLK * JPC  # 320 pairwise columns per core
NEX = 3  # exchanged colsum groups (k = 1, 2, 3)
CEX = NEX * JPC  # 192 exchanged columns (local cols 64..256)

_NC_CACHE = {}


def build_nc():
    import contextlib

    import concourse.bass as bass
    import concourse.bacc as bacc
    import concourse.mybir as mybir
    from concourse.tile import TileContext

    nc = bacc.Bacc(None, target_bir_lowering=False, debug=True)

    inT = nc.declare_dram_parameter("inT", [F, W], mybir.dt.float16, isOutput=False)
    Tm = nc.declare_dram_parameter("Tm", [F, KD], mybir.dt.float16, isOutput=False)
    # dmat[5m+d, m] = 2.0 (d-sum of 2*relu), dmat[5m+d, 32+m] = 1.0 (S row sums)
    dmat = nc.declare_dram_parameter(
        "dmat", [CHUNK, 64], mybir.dt.float16, isOutput=False
    )
    negI = nc.declare_dram_parameter("negI", [128, 128], mybir.dt.float16, isOutput=False)
    rowsum = nc.declare_dram_parameter("rowsum", [128, JPC], mybir.dt.float32, isOutput=True)
    colout = nc.declare_dram_parameter("colout", [128, CEX], mybir.dt.float32, isOutput=True)

    with TileContext(nc) as tc:
        with tc.tile_pool(name="persist", bufs=1) as pp:
            T_sb = pp.tile([128, 8 * KD], mybir.dt.float16, name="T_sb")
            inT_sb = pp.tile([128, 8 * W], mybir.dt.float16, name="inT_sb")
            dmat_sb = pp.tile([CHUNK, 64], mybir.dt.float16, name="dmat_sb")
            S16_sb = pp.tile([128, W], mybir.dt.float16, name="S16_sb")
            negSj_sb = pp.tile([128, JPC], mybir.dt.float32, name="negSj_sb")
            colacc_sb = pp.tile([128, CEX], mybir.dt.float32, name="colacc_sb")
            negI_sb = pp.tile([128, 128], mybir.dt.float16, name="negI_sb")
            xT_sb = pp.tile([128, NCHUNK * W], mybir.dt.float16, name="xT_sb")
            # f32 upcasts of xT columns 0..JPC (tensor_scalar per-partition
            # scalars must be f32). Upcast from the fp16 xT so the diagonal
            # |x - x| stays exactly zero.
            xTj_sb = pp.tile([128, NCHUNK * JPC], mybir.dt.float32, name="xTj_sb")
            raw_sb = pp.tile([128, JPC], mybir.dt.float32, name="raw_sb")

            # warm the ACT exp table while DMAs run (table load ~1.3us)
            warm_sb = pp.tile([1, 1], mybir.dt.float32, name="warm_sb")
            nc.vector.memset(warm_sb[:, :], 0.0)
            nc.scalar.activation(
                warm_sb[:, :], warm_sb[:, :], mybir.ActivationFunctionType.Exp
            )

            # Static rings: every (j, chunk) gets its own ab tile and every
            # j its own dump slot modulo 8 — cross-iteration WAW deps are
            # either absent (ab) or satisfied 8 iterations early (dump).
            ab_ring = [
                pp.tile([CHUNK, W], mybir.dt.float16, name=f"ab{t}")
                for t in range(JPC * NCHUNK)
            ]
            NDUMP = 8
            dump_ring = [
                pp.tile([128, W], mybir.dt.float16, name=f"dump{t}")
                for t in range(NDUMP)
            ]

            # --- load inputs: 2 halves each of T/inT so matmuls can start
            # after the first halves land, in 6 total strided DMAs ---
            # T halves on the SP queue, inT halves on the (otherwise idle
            # until the main loop) Pool queue, small consts on ACT after the
            # table load: the cost of a DMA is charged to its issuing queue,
            # so spreading them lands all inputs by ~3.2us instead of ~6.8us
            for h in range(2):
                nc.sync.dma_start(
                    out=T_sb[:, h * 4 * KD : (h + 1) * 4 * KD].rearrange(
                        "p (t k) -> p t k", t=4
                    ),
                    in_=Tm[h * 512 : (h + 1) * 512, :].rearrange(
                        "(t p) k -> p t k", t=4
                    ),
                )
                nc.gpsimd.dma_start(
                    out=inT_sb[:, h * 4 * W : (h + 1) * 4 * W].rearrange(
                        "p (t w) -> p t w", t=4
                    ),
                    in_=inT[h * 512 : (h + 1) * 512, :].rearrange(
                        "(t p) w -> p t w", t=4
                    ),
                )
            nc.scalar.dma_start(out=dmat_sb[:, :], in_=dmat[:, :])
            nc.scalar.dma_start(out=negI_sb[:, :], in_=negI[:, :])
            # zero the Pool-side column-sum accumulator while DMAs run
            nc.gpsimd.memset(colacc_sb[:, :], 0.0)

            with tc.tile_pool(name="xtps", bufs=2, space="PSUM") as xtps:
                # --- xT chunks: xT[kd, i] via PE over f tiles ---
                for c in range(NCHUNK):
                    xt_ps = xtps.tile([CHUNK, W], mybir.dt.float32, name="xt_ps")
                    for t in range(8):
                        nc.tensor.matmul(
                            xt_ps[:, :],
                            T_sb[:, t * KD + c * CHUNK : t * KD + (c + 1) * CHUNK],
                            inT_sb[:, t * W : (t + 1) * W],
                            start=(t == 0),
                            stop=(t == 7),
                        )
                    # alternate the PSUM->SBUF fp16 copies between ACT and
                    # DVE so the input stage drains faster
                    if c % 2 == 0:
                        nc.scalar.copy(xT_sb[0:CHUNK, c * W : (c + 1) * W], xt_ps[:, :])
                    else:
                        nc.vector.tensor_copy(
                            xT_sb[0:CHUNK, c * W : (c + 1) * W], xt_ps[:, :]
                        )
                    nc.vector.tensor_copy(
                        xTj_sb[0:CHUNK, c * JPC : (c + 1) * JPC],
                        xT_sb[0:CHUNK, c * W : c * W + JPC],
                    )

                # --- S[k, i] = sum_d x[i,k,d] at partitions 32c+m ---
                S_ps = xtps.tile([128, W], mybir.dt.float32, name="S_ps", bufs=1)
                for c in range(NCHUNK):
                    nc.tensor.matmul(
                        S_ps[32 * c : 32 * c + 32, :],
                        dmat_sb[:, 32:64],
                        xT_sb[0:CHUNK, c * W : (c + 1) * W],
                        start=True,
                        stop=True,
                        tile_position=(0, 32 * c),
                    )
                nc.vector.tensor_copy(S16_sb[:, :], S_ps[:, :])
                # exp bias column: -S_j, upcast from the SAME fp16 S16 the
                # negI matmul reads so the diagonal cancels exactly
                nc.vector.tensor_scalar(
                    negSj_sb[:, :],
                    S16_sb[:, 0:JPC],
                    -1.0,
                    0.0,
                    mybir.AluOpType.mult,
                    mybir.AluOpType.bypass,
                )

            mainps_es = contextlib.ExitStack()
            mainps = mainps_es.enter_context(
                tc.tile_pool(name="mainps", bufs=1, space="PSUM")
            )
            NDIST = 6
            dist_bufs = [
                mainps.tile([128, W], mybir.dt.float32, name=f"dist{i}")
                for i in range(NDIST)
            ]

            # --- main loop over output rows ---
            for j in range(JPC):
                dist = dist_bufs[j % NDIST]
                # dist = -S[k, i] (also absorbs the WAR wait vs the ACT exp
                # that last read this dist buffer)
                nc.tensor.matmul(
                    dist[:, :],
                    negI_sb[:, :],
                    S16_sb[:, :],
                    start=True,
                    stop=False,
                    skip_group_check=True,
                )
                for c in range(NCHUNK):
                    ab = ab_ring[j * NCHUNK + c]
                    # ab = relu(xT[:, i] - xT[:, j]) : (in - s1) max 0.0
                    # (const scalar2 keeps the second DVE read port free so
                    # the 4x perf mode applies)
                    nc.vector.tensor_scalar(
                        ab[:, :],
                        xT_sb[0:CHUNK, c * W : (c + 1) * W],
                        xTj_sb[0:CHUNK, c * JPC + j : c * JPC + j + 1],
                        0.0,
                        mybir.AluOpType.subtract,
                        mybir.AluOpType.max,
                    )
                    # dist[32c+m, :] += 2 * sum_d ab[5m+d, :]
                    nc.tensor.matmul(
                        dist[32 * c : 32 * c + 32, :],
                        dmat_sb[:, 0:32],
                        ab[:, :],
                        start=False,
                        stop=(c == NCHUNK - 1),
                        tile_position=(0, 32 * c),
                        skip_group_check=True,
                    )
                # dump = exp(-dist - S_j) = exp(-L1(i,j)) fp16;
                # accum_out gives the row sums
                nc.scalar.activation(
                    dump_ring[j % NDUMP][:, :],
                    dist[:, :],
                    mybir.ActivationFunctionType.Exp,
                    bias=negSj_sb[:, j : j + 1],
                    scale=-1.0,
                    accum_out=raw_sb[:, j : j + 1],
                )
                if j == 31:
                    # first half of the row sums is final: overlap its DMA
                    nc.sync.dma_start(out=rowsum[:, 0:32], in_=raw_sb[:, 0:32])
                # column sums on the otherwise-idle Pool (GPSIMD) engine:
                # colacc += dump[:, 64:256] (f32 accumulator in SBUF) —
                # keeps the k=1..3 colsum work entirely off PE/ACT/DVE
                nc.gpsimd.tensor_tensor(
                    colacc_sb[:, :],
                    colacc_sb[:, :],
                    dump_ring[j % NDUMP][:, JPC : JPC + CEX],
                    mybir.AluOpType.add,
                )

            mainps_es.close()
            nc.sync.dma_start(out=rowsum[:, 32:JPC], in_=raw_sb[:, 32:JPC])
            nc.gpsimd.dma_start(out=colout[:, :], in_=colacc_sb[:, :])

    nc.finalize()
    return nc


def _aux_consts():
    dm = np.zeros([CHUNK, 64], dtype=np.float16)
    for m in range(KPC):
        dm[5 * m : 5 * m + 5, m] = 2.0
        dm[5 * m : 5 * m + 5, 32 + m] = 1.0
    negI = (-np.eye(128)).astype(np.float16)
    return dm, negI


def make_in_maps(inputs, T):
    f16 = np.float16
    Tm = np.asarray(T, dtype=np.float32).astype(f16)
    dm, negI = _aux_consts()
    in_maps = []
    x = np.asarray(inputs, dtype=np.float32)
    for c in range(NCORES):
        rolled = np.roll(x, -JPC * c, axis=0)[0:W, :]
        inTc = np.ascontiguousarray(rolled.T).astype(f16)
        in_maps.append(
            {
                "inT": inTc,
                "Tm": Tm,
                "dmat": dm,
                "negI": negI,
            }
        )
    return in_maps


def assemble_output(results):
    out = np.zeros([B, K], dtype=np.float32)
    # own row sums: raw[32c+m, j] -> out[64q+j, 25c+m]
    for q in range(NCORES):
        raw = np.asarray(results[q]["rowsum"], dtype=np.float32)  # [128, JPC]
        for cc in range(NCHUNK):
            out[JPC * q : JPC * (q + 1), KPC * cc : KPC * (cc + 1)] = raw[
                32 * cc : 32 * cc + KPC, :
            ].T
    # exchanged column sums: core b's group k (k=1..3) serves rows of b+k
    for b in range(NCORES):
        col = np.asarray(results[b]["colout"], dtype=np.float32)  # [128, CEX]
        for k in range(1, NEX + 1):
            q = (b + k) % NCORES
            blk = col[:, JPC * (k - 1) : JPC * k]  # [128, JPC]
            for cc in range(NCHUNK):
                out[JPC * q : JPC * (q + 1), KPC * cc : KPC * (cc + 1)] += blk[
                    32 * cc : 32 * cc + KPC, :
                ].T
    return out


def kernel(inputs, T):
    from concourse.bass_utils import run_bass_kernel_spmd

    if "nc" not in _NC_CACHE:
        _NC_CACHE["nc"] = build_nc()
    nc = _NC_CACHE["nc"]
    in_maps = make_in_maps(inputs, T)
    res = run_bass_kernel_spmd(nc, in_maps, list(range(NCORES)))
    return assemble_output(res.results)


if __name__ == "__main__":
    sys.path.insert(0, "/root/problem")
    from reference import setup_inputs, reference

    inputs = setup_inputs()
    expected = np.asarray(reference(**inputs))
    actual = kernel(**{k: np.asarray(v) for k, v in inputs.items()})
    err = np.abs(actual - expected)
    rel = np.linalg.norm(actual - expected) / np.linalg.norm(expected)
    print(f"max abs err: {err.max():.3e}")
    print(f"Relative error: {rel:.3e}")


# revision 8
# speedup vs baseline: 1.6074x; 1.0559x over previous
"""
MinibatchDiscrimination kernel for 8x TRN2 NeuronCores (Bass/Tile).

Math:  x = inputs @ T  -> [B, K, D] with B=512, K=100, D=5
       out[i,k] = sum_j exp(-sum_d |x[i,k,d]-x[j,k,d]|)

Strategy — symmetric block-tournament over the pairwise matrix:

  The B x B pairwise matrix is tiled into 8x8 blocks of 64x64 (one row-group
  per core). Each unordered block-pair only needs computing once: from one
  computed block, ROW sums come from the ACT accumulator and COLUMN sums
  (= row sums of the transposed block, by symmetry of the L1 distance) come
  from a PE identity-matmul accumulation over the exp tiles. Core c computes
  blocks (c, c+k) for k=0..4 (mod 8, W=320 columns of its rolled copy):

    - diag block (k=0): row sums only (colsum would double-count by symmetry)
    - k=1,2,3: row sums kept locally + column sums exchanged to core c+k
      (exchange happens on the host during output assembly)
    - k=4: row sums only; the mirror pair {c, c+4} is computed independently
      by core c+4 as ITS k=4 block (distance-4 blocks are duplicated so the
      SPMD program stays identical across cores)

  Row j of core q then receives: own row sums (col-groups q..q+4) plus
  exchanged column sums from cores q-1, q-2, q-3 — all 8 groups exactly once.

Per core c of 8 (rolled by 64c so the program is SPMD-identical):
  - xT[kd, i] = sum_f T[f, kd] * inT[f, i] on PE (4 chunks of 125 kd), i<320.
  - Per output row j in 0..63:
      ab_c[p, i] = |xT_c[p, i] - xT_c[p, j]|   (DVE tensor_scalar
                   (subtract, abs_max vs 0.0) — fp16, 4x perf mode;
                   the per-partition scalar is an f32 upcast of the fp16 xT
                   column so the diagonal is exactly 0)
      dist[32c+m, :] = sum_d ab[5m+d, :]       (PE d-sum matmul with a
                   0/1 block matrix, col-tiled per chunk — no S-term or
                   negI matmul needed since abs values sum directly)
      dump[:, :]  = exp(-dist), fp16 -> SBUF   (ACT, accum_out gives the
                   row sums over all 320 cols in one pass)
      colacc     += dump[:, 64:256]            (PE identity matmul
                   accumulating in PSUM across all 64 j — the k=1,2,3
                   column sums, emitted 2 iterations late to pipeline)
  - dist row p=32c+m holds k=25c+m (m<25); host transposes/reassembles and
    adds the exchanged column-sum blocks.

  Hardware notes (CoreSim cost model, validated on TRN2 previously):
  - Steady state is ACT-bound: exp main pass 0.833*320+185 = 452ns plus the
    fixed 287ns accumulator-read = ~739ns/row; DVE 4x tensor_scalars at
    143.8ns = 575ns/row and PE 4 d-sums + colacc = 613ns/row overlap under
    it. 64 rows -> ~47us steady.
  - ab/dump tiles are STATIC rings sized to the whole loop (256 ab tiles,
    ~160KB of SBUF) so there are no cross-iteration WAW deps at all: DVE
    instructions carry no waits in steady state (the baseline lost ~10us+
    to 242 same-engine WAW EventSemaphores from rotating small rings).
  - Inputs land in 4 DMAs (two ~0.5-1MB strided transfers each for T/inT
    halves) so SP descriptor-gen time stays off the critical path; the ACT
    exp table is pre-warmed during the DMAs.
"""

import sys
import numpy as np

for _p in ("/opt/trn_rl_repo",):
    if _p not in sys.path:
        sys.path.insert(0, _p)

B = 512
F = 1024
K = 100
D = 5
KD = K * D  # 500
NCORES = 8
JPC = B // NCORES  # 64 output rows per core
NCHUNK = 4  # kd chunks of 125
CHUNK = KD // NCHUNK  # 125
KPC = K // NCHUNK  # 25 k's per chunk
NBLK = 5  # col block-groups computed per core (k = 0..4)
W = NBLK * JPC  # 320 pairwise columns per core
NEX = 3  # exchanged colsum groups (k = 1, 2, 3)
CEX = NEX * JPC  # 192 exchanged columns (local cols 64..256)

_NC_CACHE = {}


def build_nc():
    import contextlib

    import concourse.bass as bass
    import concourse.bacc as bacc
    import concourse.mybir as mybir
    from concourse.tile import TileContext

    nc = bacc.Bacc(None, target_bir_lowering=False, debug=True)

    inT = nc.declare_dram_parameter("inT", [F, W], mybir.dt.float16, isOutput=False)
    Tm = nc.declare_dram_parameter("Tm", [F, KD], mybir.dt.float16, isOutput=False)
    # dmat[5m+d, m] = 2.0 (d-sum of 2*relu), dmat[5m+d, 32+m] = 1.0 (S row sums)
    dmat = nc.declare_dram_parameter(
        "dmat", [CHUNK, 64], mybir.dt.float16, isOutput=False
    )
    negI = nc.declare_dram_parameter("negI", [128, 128], mybir.dt.float16, isOutput=False)
    rowsum = nc.declare_dram_parameter("rowsum", [128, JPC], mybir.dt.float32, isOutput=True)
    colout = nc.declare_dram_parameter(
        "colout", [128, JPC + CEX], mybir.dt.float32, isOutput=True
    )

    with TileContext(nc) as tc:
        with tc.tile_pool(name="persist", bufs=1) as pp:
            T_sb = pp.tile([128, 8 * KD], mybir.dt.float16, name="T_sb")
            inT_sb = pp.tile([128, 8 * W], mybir.dt.float16, name="inT_sb")
            dmat_sb = pp.tile([CHUNK, 64], mybir.dt.float16, name="dmat_sb")
            S16_sb = pp.tile([128, W], mybir.dt.float16, name="S16_sb")
            negSj_sb = pp.tile([128, JPC], mybir.dt.float32, name="negSj_sb")
            colacc_sb = pp.tile([128, JPC + CEX], mybir.dt.float32, name="colacc_sb")
            negI_sb = pp.tile([128, 128], mybir.dt.float16, name="negI_sb")
            xT_sb = pp.tile([128, NCHUNK * W], mybir.dt.float16, name="xT_sb")
            # f32 upcasts of xT columns 0..JPC (tensor_scalar per-partition
            # scalars must be f32). Upcast from the fp16 xT so the diagonal
            # |x - x| stays exactly zero.
            xTj_sb = pp.tile([128, NCHUNK * JPC], mybir.dt.float32, name="xTj_sb")
            raw_sb = pp.tile([128, JPC], mybir.dt.float32, name="raw_sb")

            # warm the ACT exp table while DMAs run (table load ~1.3us)
            warm_sb = pp.tile([1, 1], mybir.dt.float32, name="warm_sb")
            nc.vector.memset(warm_sb[:, :], 0.0)
            nc.scalar.activation(
                warm_sb[:, :], warm_sb[:, :], mybir.ActivationFunctionType.Exp
            )

            # Static rings: every (j, chunk) gets its own ab tile and every
            # j its own dump slot modulo 8 — cross-iteration WAW deps are
            # either absent (ab) or satisfied 8 iterations early (dump).
            ab_ring = [
                pp.tile([CHUNK, W - (t // NCHUNK)], mybir.dt.float16, name=f"ab{t}")
                for t in range(JPC * NCHUNK)
            ]
            NDUMP = 8
            dump_ring = [
                pp.tile([128, W], mybir.dt.float16, name=f"dump{t}")
                for t in range(NDUMP)
            ]

            # --- load inputs: 2 halves each of T/inT so matmuls can start
            # after the first halves land, in 6 total strided DMAs ---
            # T halves on the SP queue, inT halves on the (otherwise idle
            # until the main loop) Pool queue, small consts on ACT after the
            # table load: the cost of a DMA is charged to its issuing queue,
            # so spreading them lands all inputs by ~3.2us instead of ~6.8us
            for h in range(2):
                nc.sync.dma_start(
                    out=T_sb[:, h * 4 * KD : (h + 1) * 4 * KD].rearrange(
                        "p (t k) -> p t k", t=4
                    ),
                    in_=Tm[h * 512 : (h + 1) * 512, :].rearrange(
                        "(t p) k -> p t k", t=4
                    ),
                )
                nc.gpsimd.dma_start(
                    out=inT_sb[:, h * 4 * W : (h + 1) * 4 * W].rearrange(
                        "p (t w) -> p t w", t=4
                    ),
                    in_=inT[h * 512 : (h + 1) * 512, :].rearrange(
                        "(t p) w -> p t w", t=4
                    ),
                )
            nc.scalar.dma_start(out=dmat_sb[:, :], in_=dmat[:, :])
            nc.scalar.dma_start(out=negI_sb[:, :], in_=negI[:, :])
            # zero the Pool-side column-sum accumulator while DMAs run
            nc.gpsimd.memset(colacc_sb[:, :], 0.0)

            with tc.tile_pool(name="xtps", bufs=2, space="PSUM") as xtps:
                # --- xT chunks: xT[kd, i] via PE over f tiles ---
                for c in range(NCHUNK):
                    xt_ps = xtps.tile([CHUNK, W], mybir.dt.float32, name="xt_ps")
                    for t in range(8):
                        nc.tensor.matmul(
                            xt_ps[:, :],
                            T_sb[:, t * KD + c * CHUNK : t * KD + (c + 1) * CHUNK],
                            inT_sb[:, t * W : (t + 1) * W],
                            start=(t == 0),
                            stop=(t == 7),
                        )
                    # alternate the PSUM->SBUF fp16 copies between ACT and
                    # DVE so the input stage drains faster
                    if c % 2 == 0:
                        nc.scalar.copy(xT_sb[0:CHUNK, c * W : (c + 1) * W], xt_ps[:, :])
                    else:
                        nc.vector.tensor_copy(
                            xT_sb[0:CHUNK, c * W : (c + 1) * W], xt_ps[:, :]
                        )
                    nc.vector.tensor_copy(
                        xTj_sb[0:CHUNK, c * JPC : (c + 1) * JPC],
                        xT_sb[0:CHUNK, c * W : c * W + JPC],
                    )

                # --- S[k, i] = sum_d x[i,k,d] at partitions 32c+m ---
                S_ps = xtps.tile([128, W], mybir.dt.float32, name="S_ps", bufs=1)
                for c in range(NCHUNK):
                    nc.tensor.matmul(
                        S_ps[32 * c : 32 * c + 32, :],
                        dmat_sb[:, 32:64],
                        xT_sb[0:CHUNK, c * W : (c + 1) * W],
                        start=True,
                        stop=True,
                        tile_position=(0, 32 * c),
                    )
                nc.vector.tensor_copy(S16_sb[:, :], S_ps[:, :])
                # exp bias column: -S_j, upcast from the SAME fp16 S16 the
                # negI matmul reads so the diagonal cancels exactly
                nc.vector.tensor_scalar(
                    negSj_sb[:, :],
                    S16_sb[:, 0:JPC],
                    -1.0,
                    0.0,
                    mybir.AluOpType.mult,
                    mybir.AluOpType.bypass,
                )

            mainps_es = contextlib.ExitStack()
            mainps = mainps_es.enter_context(
                tc.tile_pool(name="mainps", bufs=1, space="PSUM")
            )
            NDIST = 6
            dist_bufs = [
                mainps.tile([128, W], mybir.dt.float32, name=f"dist{i}")
                for i in range(NDIST)
            ]

            # --- main loop over output rows ---
            # Row j only computes columns i >= j (ragged upper triangle):
            # the diagonal block's lower-triangle contributions come from the
            # colacc by symmetry (minus the double-counted self term 1.0,
            # subtracted on the host).
            CEND = JPC + CEX  # colacc covers cols 0..256
            for j in range(JPC):
                dist = dist_bufs[j % NDIST]
                # dist = -S[k, i] (also absorbs the WAR wait vs the ACT exp
                # that last read this dist buffer)
                nc.tensor.matmul(
                    dist[:, j:W],
                    negI_sb[:, :],
                    S16_sb[:, j:W],
                    start=True,
                    stop=False,
                    skip_group_check=True,
                )
                for c in range(NCHUNK):
                    ab = ab_ring[j * NCHUNK + c]
                    # ab = relu(xT[:, i] - xT[:, j]) : (in - s1) max 0.0
                    # (const scalar2 keeps the second DVE read port free so
                    # the 4x perf mode applies)
                    nc.vector.tensor_scalar(
                        ab[:, :],
                        xT_sb[0:CHUNK, c * W + j : (c + 1) * W],
                        xTj_sb[0:CHUNK, c * JPC + j : c * JPC + j + 1],
                        0.0,
                        mybir.AluOpType.subtract,
                        mybir.AluOpType.max,
                    )
                    # dist[32c+m, :] += 2 * sum_d ab[5m+d, :]
                    nc.tensor.matmul(
                        dist[32 * c : 32 * c + 32, j:W],
                        dmat_sb[:, 0:32],
                        ab[:, :],
                        start=False,
                        stop=(c == NCHUNK - 1),
                        tile_position=(0, 32 * c),
                        skip_group_check=True,
                    )
                # dump = exp(-dist - S_j) = exp(-L1(i,j)) fp16;
                # accum_out gives the row sums over cols j..320
                nc.scalar.activation(
                    dump_ring[j % NDUMP][:, j:W],
                    dist[:, j:W],
                    mybir.ActivationFunctionType.Exp,
                    bias=negSj_sb[:, j : j + 1],
                    scale=-1.0,
                    accum_out=raw_sb[:, j : j + 1],
                )
                if j == 31:
                    # first half of the row sums is final: overlap its DMA
                    nc.sync.dma_start(out=rowsum[:, 0:32], in_=raw_sb[:, 0:32])
                # column sums on the otherwise-idle Pool (GPSIMD) engine:
                # colacc += dump[:, j:256] (f32 accumulator in SBUF) — the
                # diag block part doubles as the lower-triangle row sums
                nc.gpsimd.tensor_tensor(
                    colacc_sb[:, j:CEND],
                    colacc_sb[:, j:CEND],
                    dump_ring[j % NDUMP][:, j:CEND],
                    mybir.AluOpType.add,
                )

            mainps_es.close()
            nc.sync.dma_start(out=rowsum[:, 32:JPC], in_=raw_sb[:, 32:JPC])
            nc.gpsimd.dma_start(out=colout[:, :], in_=colacc_sb[:, :])

    nc.finalize()
    return nc


def _aux_consts():
    dm = np.zeros([CHUNK, 64], dtype=np.float16)
    for m in range(KPC):
        dm[5 * m : 5 * m + 5, m] = 2.0
        dm[5 * m : 5 * m + 5, 32 + m] = 1.0
    negI = (-np.eye(128)).astype(np.float16)
    return dm, negI


def make_in_maps(inputs, T):
    f16 = np.float16
    Tm = np.asarray(T, dtype=np.float32).astype(f16)
    dm, negI = _aux_consts()
    in_maps = []
    x = np.asarray(inputs, dtype=np.float32)
    for c in range(NCORES):
        rolled = np.roll(x, -JPC * c, axis=0)[0:W, :]
        inTc = np.ascontiguousarray(rolled.T).astype(f16)
        in_maps.append(
            {
                "inT": inTc,
                "Tm": Tm,
                "dmat": dm,
                "negI": negI,
            }
        )
    return in_maps


def assemble_output(results):
    out = np.zeros([B, K], dtype=np.float32)
    # own row sums: raw[32c+m, j] -> out[64q+j, 25c+m]
    for q in range(NCORES):
        raw = np.asarray(results[q]["rowsum"], dtype=np.float32)  # [128, JPC]
        for cc in range(NCHUNK):
            out[JPC * q : JPC * (q + 1), KPC * cc : KPC * (cc + 1)] = raw[
                32 * cc : 32 * cc + KPC, :
            ].T
    # column sums: core b's group k serves rows of core b+k. k=0 is the own
    # diag block (lower triangle by symmetry; subtract the double-counted
    # self term exp(0)=1), k=1..3 are the exchanged off-diag groups.
    for b in range(NCORES):
        col = np.asarray(results[b]["colout"], dtype=np.float32)  # [128, JPC+CEX]
        for k in range(0, NEX + 1):
            q = (b + k) % NCORES
            blk = col[:, JPC * k : JPC * (k + 1)]  # [128, JPC]
            for cc in range(NCHUNK):
                out[JPC * q : JPC * (q + 1), KPC * cc : KPC * (cc + 1)] += blk[
                    32 * cc : 32 * cc + KPC, :
                ].T
    out -= 1.0
    return out


def kernel(inputs, T):
    from concourse.bass_utils import run_bass_kernel_spmd

    if "nc" not in _NC_CACHE:
        _NC_CACHE["nc"] = build_nc()
    nc = _NC_CACHE["nc"]
    in_maps = make_in_maps(inputs, T)
    res = run_bass_kernel_spmd(nc, in_maps, list(range(NCORES)))
    return assemble_output(res.results)


if __name__ == "__main__":
    sys.path.insert(0, "/root/problem")
    from reference import setup_inputs, reference

    inputs = setup_inputs()
    expected = np.asarray(reference(**inputs))
    actual = kernel(**{k: np.asarray(v) for k, v in inputs.items()})
    err = np.abs(actual - expected)
    rel = np.linalg.norm(actual - expected) / np.linalg.norm(expected)
    print(f"max abs err: {err.max():.3e}")
    print(f"Relative error: {rel:.3e}")
